# revision 23
# baseline (speedup 1.0000x reference)
"""Trainium2 Bass kernel for LocalGlobalSelfAttention (fp8 DoubleRow).

Sharding: 8 cores = 4 batches x 2 sequence-halves (no collectives).
Each core computes, for its (batch b, half h):
  - global attention: queries = its half (SH rows), keys/values = full seq
  - local windowed attention: fully contained in its half
  - output projections (g+l accumulated in PSUM) + residual + layernorm

All matmuls run fp8e4 with fp32 PSUM accumulation; contraction-128 matmuls
(projections, AV, output projection) use DoubleRow (2x effective throughput).
Scores (K=64 contraction) run plain fp8. Scaling ledger: weights host-scaled
x32 (fp8 subnormal avoidance), un-scaled in the PSUM->SBUF cast; attention
outputs carry x64 (ones-column = 1/64, so the rowsum reciprocal yields
64/sum); the final combine divides by 64*32 = 2048. Softmax skips the max
subtraction (scores are O(1)); rowsums ride the AV matmul via the ones
column (placed FIRST so the rowsum lands in psum partition 0 - the custom
DVE reciprocal ignores AP partition offsets); v dims sit at columns 64-127
(64-partition reads must start at partition 0 or 64).

Pipeline: the global-attention phase is scalar(exp)-bound, so all deferrable
tensor work - v projections, the six local projection sets, local attention
heads, and out-proj operand DMA loads - is emitted as fill units between
score/exp pairs of the global heads (emission order matters: engine queues
are in-order FIFOs, so every dependency of an instruction must be emitted
before it).
"""

import numpy as np
import ml_dtypes
from collections import deque
from contextlib import ExitStack

F8 = ml_dtypes.float8_e4m3

FULL_CFG = dict(S=2048, D=1024, H=16, K=64, NW=8)
N_CORES = 8
LN_EPS = 1e-3
WSCALE = 32.0    # host weight pre-scale
OSCALE = 64.0    # attention-output scale (ones column = 1/OSCALE)


def _chunks(total, size):
    return [(o, min(size, total - o)) for o in range(0, total, size)]


def flags_for(inputs):
    def nz(*keys):
        return any(np.any(np.asarray(inputs[k])) for k in keys)
    return dict(
        bqk_g=nz("gbq", "gbk"), bqk_l=nz("lbq", "lbk"),
        bv_g=nz("gbv"), bv_l=nz("lbv"),
        ln=(not np.all(np.asarray(inputs["gamma"]) == 1.0)
            or np.any(np.asarray(inputs["beta"]))),
    )


DEFAULT_FLAGS = dict(bqk_g=False, bqk_l=False, bv_g=False, bv_l=False,
                     ln=False)


def build_nc(cfg=None, flags=None):
    """Build + compile the per-core Bass program (SPMD, same on all cores)."""
    import concourse.bass as bass
    import concourse.tile as tile
    import concourse.mybir as mybir
    from concourse import bacc

    cfg = dict(cfg or FULL_CFG)
    flags = dict(flags or DEFAULT_FLAGS)
    S, D, H, K, NW = cfg["S"], cfg["D"], cfg["H"], cfg["K"], cfg["NW"]
    HK = H * K
    SH = S // 2          # per-core query rows (half the sequence)
    WIN = S // NW        # local attention window
    NWH = SH // WIN      # windows in this core's half
    assert K == 64 and D % 256 == 0 and HK % 128 == 0
    assert SH % 128 == 0 and WIN % 128 == 0 and NWH * WIN == SH

    ND2 = D // 256       # d-pair tiles (DoubleRow contracts 256 at a time)
    NHK = HK // 128      # head-pair tiles (2 heads each)
    NST = S // 128       # s-tiles (full seq)
    NSP = NST // 2       # s-tile pairs (global)
    NQT = SH // 128      # q-tiles (half seq)
    NLP = SH // 256      # local s-tile pairs = windows in half
    NOT = NHK // 2       # hk-pair tiles for output proj

    f32 = mybir.dt.float32
    fp8 = mybir.dt.float8e4
    DR = mybir.MatmulPerfMode.DoubleRow
    Exp = mybir.ActivationFunctionType.Exp
    Square = mybir.ActivationFunctionType.Square
    Sqrt = mybir.ActivationFunctionType.Sqrt
    add_op = mybir.AluOpType.add
    mult_op = mybir.AluOpType.mult
    sub_op = mybir.AluOpType.subtract

    nc = bacc.Bacc("TRN2", target_bir_lowering=False, debug=False,
                   num_devices=N_CORES)

    # ---- DRAM parameters -------------------------------------------------
    xT2_d = nc.dram_tensor("xT2", [ND2, 128, 2, S], fp8, kind="ExternalInput")
    xq_d = nc.dram_tensor("xq", [SH, D], f32, kind="ExternalInput")
    w_d = {}
    for nm in ("wq_g", "wk_g", "wv_g", "wq_l", "wk_l", "wv_l"):
        w_d[nm] = nc.dram_tensor(nm, [ND2, 128, 2, HK], fp8,
                                 kind="ExternalInput")
    wo_d = {st: nc.dram_tensor(f"wo_{st}", [NOT, 128, 2, D], fp8,
                               kind="ExternalInput") for st in ("g", "l")}
    bcol_d = {}
    for st in ("g", "l"):
        if flags[f"bqk_{st}"]:
            for qk in ("q", "k"):
                bcol_d[f"b{qk}_{st}"] = nc.dram_tensor(
                    f"b{qk}_{st}", [NHK, 128], f32, kind="ExternalInput")
        if flags[f"bv_{st}"]:
            bcol_d[f"bv_{st}"] = nc.dram_tensor(
                f"bv_{st}", [1, HK], fp8, kind="ExternalInput")
    if flags["ln"]:
        gamma_d = nc.dram_tensor("gamma", [1, D], f32, kind="ExternalInput")
        beta_d = nc.dram_tensor("beta", [1, D], f32, kind="ExternalInput")
    out_d = nc.dram_tensor("out", [SH, D], f32, kind="ExternalOutput")

    # DRAM scratch for attention outputs, laid out for the out-proj DoubleRow
    # load: [t, p, j, q] with hk = 256 t + 128 j + p
    oscr = {st: nc.dram_tensor(f"oscr_{st}", [NOT, 128, 2, SH], fp8)
            for st in ("g", "l")}

    PS = bass.MemorySpace.PSUM

    with tile.TileContext(nc) as tc, ExitStack() as ctx:
        # ---- whole-life pools -------------------------------------------
        cpool = ctx.enter_context(tc.tile_pool(name="consts", bufs=1))
        wopool = ctx.enter_context(tc.tile_pool(name="wo", bufs=1))
        odpool = ctx.enter_context(tc.tile_pool(name="od", bufs=1))

        ones8 = cpool.tile([1, 128], fp8, tag="ones", name="ones")
        nc.vector.memset(ones8[:], 1.0)
        eps_col = cpool.tile([128, 1], f32, tag="eps", name="eps")
        nc.vector.memset(eps_col[:], float(LN_EPS))
        bcol_sb = {}
        for nm, d in bcol_d.items():
            if nm.startswith("bv"):
                t = cpool.tile([1, HK], fp8, tag=nm, name=nm)
                nc.sync.dma_start(t[:], d[:])
                bcol_sb[nm] = t
            else:
                cols = []
                for j in range(NHK):
                    t = cpool.tile([128, 1], f32, tag=f"{nm}{j}",
                                   name=f"{nm}{j}")
                    nc.sync.dma_start(t[:], d[j, :].rearrange("(a b) -> a b",
                                                              b=1))
                    cols.append(t)
                bcol_sb[nm] = cols

        wo_sb = {}
        for st in ("g", "l"):
            wo_sb[st] = [wopool.tile([128, 2, D], fp8, tag=f"wo{st}{t}",
                                     name=f"wo{st}{t}") for t in range(NOT)]
        o_sb = {st: [odpool.tile([128, 2, SH], fp8, tag=f"ob{st}{t}",
                                 name=f"ob{st}{t}") for t in range(NOT)]
                for st in ("g", "l")}

        # ---- mid-life pools (released before phase D) --------------------
        mid_ctx = ctx.enter_context(ExitStack())
        kqv = mid_ctx.enter_context(tc.tile_pool(name="kqv", bufs=1))
        xin = mid_ctx.enter_context(tc.tile_pool(name="xin", bufs=1))
        wpool = mid_ctx.enter_context(tc.tile_pool(name="wt", bufs=1))

        kT_g = [kqv.tile([128, S], fp8, tag=f"ktg{j}", name=f"ktg{j}")
                for j in range(NHK)]
        qT_g = [kqv.tile([128, SH], fp8, tag=f"qtg{j}", name=f"qtg{j}")
                for j in range(NHK)]
        vx_g = [kqv.tile([128, 2, H, 128], fp8, tag=f"vxg{u}", name=f"vxg{u}")
                for u in range(NSP)]
        kT_l = [kqv.tile([128, SH], fp8, tag=f"ktl{j}", name=f"ktl{j}")
                for j in range(NHK)]
        qT_l = [kqv.tile([128, SH], fp8, tag=f"qtl{j}", name=f"qtl{j}")
                for j in range(NHK)]
        vx_l = [kqv.tile([128, 2, H, 128], fp8, tag=f"vxl{u}", name=f"vxl{u}")
                for u in range(NLP)]
        for u in range(NSP):
            nc.vector.memset(vx_g[u][:, :, :, 0:1], 1.0 / OSCALE)
        for u in range(NLP):
            nc.vector.memset(vx_l[u][:, :, :, 0:1], 1.0 / OSCALE)

        x2t = [xin.tile([128, 2, S], fp8, tag=f"xt{t}", name=f"xt{t}")
               for t in range(ND2)]
        w_sb = {}
        for nm in ("wk_g", "wq_g", "wv_g", "wk_l", "wq_l", "wv_l"):
            w_sb[nm] = [wpool.tile([128, 2, HK], fp8, tag=f"{nm}{t}",
                                   name=f"{nm}{t}") for t in range(ND2)]
        # DMA order: interleave x tiles with the first weight set so the
        # first projection chain starts as early as possible
        for t in range(ND2):
            nc.sync.dma_start(x2t[t][:], xT2_d[t])
            nc.sync.dma_start(w_sb["wk_g"][t][:], w_d["wk_g"][t])
        for nm in ("wq_g", "wv_g", "wk_l", "wq_l", "wv_l"):
            for t in range(ND2):
                nc.sync.dma_start(w_sb[nm][t][:], w_d[nm][t])
        for st in ("g", "l"):
            for t in range(NOT):
                nc.sync.dma_start(wo_sb[st][t][:], wo_d[st][t])

        # ================= Phase A: global K/Q projections ================
        with tc.tile_pool(name="ppA", bufs=2, space=PS) as ppA:
            def proj_kq_g(nm, s_len, out_tiles, bias):
                for j in range(NHK):
                    pt = ppA.tile([128, 2048], f32, tag="pp", name=f"p{nm}{j}")
                    for t in range(ND2):
                        for so, sl in _chunks(s_len, 512):
                            nc.tensor.matmul(
                                pt[:, so:so + sl],
                                w_sb[nm][t][:, :, j * 128:(j + 1) * 128],
                                x2t[t][:, :, so:so + sl],
                                start=(t == 0), stop=(t == ND2 - 1),
                                perf_mode=DR)
                    if bias is not None:
                        nc.vector.tensor_scalar(
                            out_tiles[j][:], pt[:, 0:s_len], 1.0 / WSCALE,
                            bias[j], mult_op, add_op)
                    else:
                        nc.vector.tensor_scalar(
                            out_tiles[j][:], pt[:, 0:s_len], 1.0 / WSCALE,
                            None, mult_op)

            proj_kq_g("wk_g", S, kT_g, bcol_sb.get("bk_g"))
            proj_kq_g("wq_g", SH, qT_g, bcol_sb.get("bq_g"))

        # ============== Phases B/C: attention + deferred fill work ========
        bc_ctx = ctx.enter_context(ExitStack())
        scp = bc_ctx.enter_context(tc.tile_pool(name="scp", bufs=2, space=PS))
        opp = bc_ctx.enter_context(tc.tile_pool(name="opp", bufs=2, space=PS))
        exp_p = bc_ctx.enter_context(tc.tile_pool(name="exp", bufs=4))
        nop = bc_ctx.enter_context(tc.tile_pool(name="nop", bufs=2))

        # --- fill units: projections into scp-pool psum tiles -------------
        def fill_kq(nm, out_tiles, bias, j):
            pt = scp.tile([128, SH], f32, tag="sc", name=f"f{nm}{j}")
            for t in range(ND2):
                for so, sl in _chunks(SH, 512):
                    nc.tensor.matmul(
                        pt[:, so:so + sl],
                        w_sb[nm][t][:, :, j * 128:(j + 1) * 128],
                        x2t[t][:, :, so:so + sl],
                        start=(t == 0), stop=(t == ND2 - 1), perf_mode=DR)
            if bias is not None:
                nc.vector.tensor_scalar(out_tiles[j][:], pt[:], 1.0 / WSCALE,
                                        bias[j], mult_op, add_op)
            else:
                nc.vector.tensor_scalar(out_tiles[j][:], pt[:], 1.0 / WSCALE,
                                        None, mult_op)

        def fill_v(nm, vx_tiles, bias_row, ts_):
            u, jj = divmod(ts_, 2)
            pt = scp.tile([128, SH], f32, tag="sc", name=f"f{nm}{ts_}")
            for t in range(ND2):
                for ho, hl in _chunks(HK, 512):
                    st_ = t == 0
                    sp_ = t == ND2 - 1 and bias_row is None
                    nc.tensor.matmul(
                        pt[:, ho:ho + hl],
                        x2t[t][:, :, ts_ * 128:(ts_ + 1) * 128],
                        w_sb[nm][t][:, :, ho:ho + hl],
                        start=st_, stop=sp_, perf_mode=DR)
            if bias_row is not None:
                for ho, hl in _chunks(HK, 512):
                    nc.tensor.matmul(pt[:, ho:ho + hl], ones8[0:1, 0:128],
                                     bias_row[0:1, ho:ho + hl],
                                     start=False, stop=True)
            nc.vector.tensor_scalar(
                vx_tiles[u][:, jj, :, 64:128],
                pt[:].rearrange("p (h k) -> p h k", k=64),
                1.0 / WSCALE, None, mult_op)

        def normalize_store(h, o_ps, dst):
            # rinv = OSCALE / rowsum ; o8 = o * rinv  (x OSCALE into fp8)
            rinv = nop.tile([1, SH], f32, tag="ri", name=f"ri{h}", bufs=1)
            nc.vector.reciprocal_approx_fast(out=rinv[:], in_=o_ps[0:1, :])
            rb = nop.tile([64, SH], f32, tag="rb", name=f"rb{h}")
            nc.gpsimd.partition_broadcast(rb[:], rinv[0:1, :])
            o8 = nop.tile([64, SH], fp8, tag="o8", name=f"o8{h}")
            nc.vector.tensor_tensor(o8[:], o_ps[64:128, :], rb[:], mult_op)
            t, j, pr = h // 4, (h % 4) // 2, 64 * (h % 2)
            nc.sync.dma_start(dst[t][pr:pr + 64, j, :], o8[:])

        def local_head(h):
            hp, po = h // 2, 64 * (h % 2)
            o_ps = opp.tile([128, SH], f32, tag="o", name=f"ol{h}")
            ex2 = exp_p.tile([128, 2, SH], fp8, tag="ex", name=f"exl{h}")
            for ss in range(2):
                sc = scp.tile([128, SH], f32, tag="sc", name=f"scl{h}{ss}")
                for w in range(NWH):
                    st = 2 * w + ss
                    nc.tensor.matmul(
                        sc[:, w * WIN:(w + 1) * WIN],
                        kT_l[hp][po:po + 64, st * 128:(st + 1) * 128],
                        qT_l[hp][po:po + 64, w * WIN:(w + 1) * WIN],
                        start=True, stop=True)
                nc.scalar.activation(ex2[:, ss, :], sc[:], Exp, scale=0.125)
            for w in range(NWH):
                nc.tensor.matmul(
                    o_ps[:, w * WIN:(w + 1) * WIN], vx_l[w][:, :, h, :],
                    ex2[:, :, w * WIN:(w + 1) * WIN],
                    start=(w % 2 == 0), stop=(w % 2 == 1), perf_mode=DR)
            normalize_store(h, o_ps, oscr["l"])

        def load_o2(st, t):
            nc.sync.dma_start(o_sb[st][t][:], oscr[st][t])

        # fill queue: (cost_estimate_us, closure); emitted between score/exp
        # pairs. Order respects the in-order engine FIFOs: everything an
        # emitted instruction depends on is emitted earlier. v_g fills are
        # kept separate: ensure_vg() guarantees the producer of vx_g[u] is
        # emitted before any AV matmul that reads it.
        vg_next = [0]

        def ensure_vg(up_to_ts):
            while vg_next[0] <= min(up_to_ts, NST - 1):
                fill_v("wv_g", vx_g, bcol_sb.get("bv_g"), vg_next[0])
                vg_next[0] += 1

        fills = deque()
        for j in range(NHK):
            fills.append((1.8, lambda j=j: fill_kq(
                "wk_l", kT_l, bcol_sb.get("bk_l"), j)))
        for j in range(NHK):
            fills.append((1.8, lambda j=j: fill_kq(
                "wq_l", qT_l, bcol_sb.get("bq_l"), j)))
        for ts_ in range(SH // 128):
            fills.append((1.8, lambda ts_=ts_: fill_v(
                "wv_l", vx_l, bcol_sb.get("bv_l"), ts_)))

        for h in range(H):
            fills.append((2.2, lambda h=h: local_head(h)))
            if h % 4 == 3:
                fills.append((0.0, lambda t=h // 4: load_o2("l", t)))

        def pop_fills(budget):
            while fills and budget > 0:
                cost, fn = fills.popleft()
                fn()
                budget -= max(cost, 0.1)

        # v_g pairs 0-1 must exist before head 0's first AV matmuls
        ensure_vg(3)

        # --- global attention, one head at a time -------------------------
        for h in range(H):
            hp, po = h // 2, 64 * (h % 2)
            o_ps = opp.tile([128, SH], f32, tag="o", name=f"og{h}")
            pend = deque()

            def do_av(item, o_ps=o_ps, h=h):
                ex2, u = item
                ensure_vg(2 * u + 1)
                for qo, ql in _chunks(SH, 512):
                    nc.tensor.matmul(
                        o_ps[:, qo:qo + ql], vx_g[u][:, :, h, :],
                        ex2[:, :, qo:qo + ql],
                        start=(u == 0), stop=(u == NSP - 1), perf_mode=DR)

            for u in range(NSP):
                ex2 = exp_p.tile([128, 2, SH], fp8, tag="ex",
                                 name=f"exg{h}{u}")
                for j in range(2):
                    st = 2 * u + j
                    sc = scp.tile([128, SH], f32, tag="sc", name=f"scg{h}{st}")
                    for qo, ql in _chunks(SH, 512):
                        nc.tensor.matmul(
                            sc[:, qo:qo + ql],
                            kT_g[hp][po:po + 64, st * 128:(st + 1) * 128],
                            qT_g[hp][po:po + 64, qo:qo + ql],
                            start=True, stop=True)
                    nc.scalar.activation(ex2[:, j, :], sc[:], Exp, scale=0.125)
                pend.append((ex2, u))
                if len(pend) > 2:
                    do_av(pend.popleft())
                pop_fills(1.0)
            while pend:
                do_av(pend.popleft())
            ensure_vg(NST - 1)
            normalize_store(h, o_ps, oscr["g"])
            if h % 4 == 3:
                load_o2("g", h // 4)

        while fills:
            cost, fn = fills.popleft()
            fn()


        # ========== Phase D: output projection + residual + layernorm ====
        bc_ctx.close()   # release attention PSUM banks for the ypp pool
        mid_ctx.close()  # release K/Q/V + x + projection-weight SBUF

        with tc.tile_pool(name="ypp", bufs=2, space=PS) as ypp, \
             tc.tile_pool(name="ln", bufs=2) as lnp:
            if flags["ln"]:
                gamma_bc = lnp.tile([128, D], f32, tag="gamma", name="gamma",
                                    bufs=1)
                nc.sync.dma_start(gamma_bc[:],
                                  gamma_d[:].partition_broadcast(128))
                beta_bc = lnp.tile([128, D], f32, tag="beta", name="beta",
                                   bufs=1)
                nc.sync.dma_start(beta_bc[:],
                                  beta_d[:].partition_broadcast(128))
            for qt in range(NQT):
                xq_t = lnp.tile([128, D], f32, tag="xq", name=f"xq{qt}")
                nc.sync.dma_start(xq_t[:], xq_d[qt * 128:(qt + 1) * 128, :])
                ps_y = ypp.tile([128, D], f32, tag="py", name=f"py{qt}")
                for do, dl in _chunks(D, 512):
                    first = True
                    for st in ("g", "l"):
                        for t in range(NOT):
                            nc.tensor.matmul(
                                ps_y[:, do:do + dl],
                                o_sb[st][t][:, :, qt * 128:(qt + 1) * 128],
                                wo_sb[st][t][:, :, do:do + dl],
                                start=first, stop=(st == "l" and t == NOT - 1),
                                perf_mode=DR)
                            first = False
                y = lnp.tile([128, D], f32, tag="y", name=f"y{qt}")
                ssum = lnp.tile([128, 1], f32, tag="ssum", name=f"ssum{qt}")
                nc.vector.scalar_tensor_tensor(
                    y[:], ps_y[:], 1.0 / (WSCALE * OSCALE), xq_t[:],
                    mult_op, add_op, accum_out=ssum[:])
                sqd = lnp.tile([128, D], f32, tag="sqd", name=f"sqd{qt}")
                ssq = lnp.tile([128, 1], f32, tag="ssq", name=f"ssq{qt}")
                nc.scalar.activation(sqd[:], y[:], Square, accum_out=ssq[:])
                mu = lnp.tile([128, 1], f32, tag="mu", name=f"mu{qt}")
                nc.vector.tensor_scalar_mul(mu[:], ssum[:], 1.0 / D)
                var = lnp.tile([128, 1], f32, tag="var", name=f"var{qt}")
                nc.vector.tensor_scalar_mul(var[:], ssq[:], 1.0 / D)
                mu2 = lnp.tile([128, 1], f32, tag="mu2", name=f"mu2{qt}")
                nc.vector.tensor_tensor(mu2[:], mu[:], mu[:], mult_op)
                nc.vector.tensor_tensor(var[:], var[:], mu2[:], sub_op)
                sd = lnp.tile([128, 1], f32, tag="sd", name=f"sd{qt}")
                nc.scalar.activation(sd[:], var[:], Sqrt, bias=eps_col[:])
                rstd = lnp.tile([128, 1], f32, tag="rstd", name=f"rstd{qt}")
                nc.vector.reciprocal(rstd[:], sd[:])
                bco = lnp.tile([128, 1], f32, tag="bco", name=f"bco{qt}")
                nc.vector.tensor_tensor(bco[:], mu[:], rstd[:], mult_op)
                nc.vector.tensor_scalar_mul(bco[:], bco[:], -1.0)
                ot = lnp.tile([128, D], f32, tag="ot", name=f"ot{qt}")
                nc.vector.tensor_scalar(ot[:], y[:], rstd[:], bco[:],
                                        mult_op, add_op)
                if flags["ln"]:
                    t2 = lnp.tile([128, D], f32, tag="t2", name=f"t2{qt}")
                    nc.vector.tensor_tensor(t2[:], ot[:], gamma_bc[:], mult_op)
                    nc.vector.tensor_tensor(ot[:], t2[:], beta_bc[:], add_op)
                nc.sync.dma_start(out_d[qt * 128:(qt + 1) * 128, :], ot[:])

    nc.compile()
    return nc


def make_in_maps(inputs, cfg=None, flags=None):
    """Build per-core input maps from the full (unsharded) problem inputs."""
    cfg = dict(cfg or FULL_CFG)
    flags = dict(flags or DEFAULT_FLAGS)
    S, D, H, K = cfg["S"], cfg["D"], cfg["H"], cfg["K"]
    HK = H * K
    SH = S // 2
    ND2 = D // 256
    NHK = HK // 128
    NOT = NHK // 2

    def np32(a):
        return np.asarray(a, dtype=np.float32)

    def dpair(w):  # [D, X] -> [ND2, 128, 2, X]
        return np.ascontiguousarray(
            w.reshape(ND2, 2, 128, -1).transpose(0, 2, 1, 3))

    shared = {}
    for nm, key in (("wq_g", "gWq"), ("wk_g", "gWk"), ("wv_g", "gWv"),
                    ("wq_l", "lWq"), ("wk_l", "lWk"), ("wv_l", "lWv")):
        shared[nm] = dpair(np32(inputs[key]).reshape(D, HK) * WSCALE).astype(F8)
    for st, key in (("g", "gWo"), ("l", "lWo")):
        w = np32(inputs[key]).reshape(HK, D) * WSCALE
        shared[f"wo_{st}"] = np.ascontiguousarray(
            w.reshape(NOT, 2, 128, D).transpose(0, 2, 1, 3)).astype(F8)
    for st, q, k in (("g", "gbq", "gbk"), ("l", "lbq", "lbk")):
        if flags[f"bqk_{st}"]:
            shared[f"bq_{st}"] = np.ascontiguousarray(
                np32(inputs[q]).reshape(NHK, 128))
            shared[f"bk_{st}"] = np.ascontiguousarray(
                np32(inputs[k]).reshape(NHK, 128))
    for st, key in (("g", "gbv"), ("l", "lbv")):
        if flags[f"bv_{st}"]:
            shared[f"bv_{st}"] = (np32(inputs[key]).reshape(1, HK)
                                  * WSCALE).astype(F8)
    if flags["ln"]:
        shared["gamma"] = np32(inputs["gamma"]).reshape(1, D)
        shared["beta"] = np32(inputs["beta"]).reshape(1, D)

    x = np32(inputs["x"])
    bo = np32(inputs["gbo"]) + np32(inputs["lbo"])
    in_maps = []
    for c in range(N_CORES):
        b, half = divmod(c, 2)
        xb = x[b]
        # own half first (queries/local), other half second; global attention
        # is invariant to key/value column order
        xperm = np.concatenate([xb[half * SH:(half + 1) * SH],
                                xb[(1 - half) * SH:(2 - half) * SH]], axis=0)
        m = dict(shared)
        m["xT2"] = dpair(np.ascontiguousarray(xperm.T)).astype(F8)
        m["xq"] = np.ascontiguousarray(xperm[0:SH]) + bo
        in_maps.append(m)
    return in_maps


def assemble_out(results, cfg=None):
    cfg = dict(cfg or FULL_CFG)
    S, D = cfg["S"], cfg["D"]
    SH = S // 2
    B = N_CORES // 2
    out = np.empty((B, S, D), np.float32)
    for c in range(N_CORES):
        b, half = divmod(c, 2)
        out[b, half * SH:(half + 1) * SH] = results[c]["out"]
    return out


_NC_CACHE = {}


def kernel(**inputs):
    from concourse.bass_utils import run_bass_kernel_spmd
    flags = flags_for(inputs)
    key = tuple(sorted(flags.items()))
    if key not in _NC_CACHE:
        _NC_CACHE[key] = build_nc(flags=flags)
    nc = _NC_CACHE[key]
    in_maps = make_in_maps(inputs, flags=flags)
    res = run_bass_kernel_spmd(nc, in_maps, list(range(N_CORES)))
    return assemble_out(res.results)


# revision 24
# speedup vs baseline: 1.1829x; 1.1829x over previous
"""Trainium2 Bass kernel for LocalGlobalSelfAttention (fp8 DoubleRow).

Sharding: 8 cores = 4 batches x 2 sequence-halves (no collectives).
Each core computes, for its (batch b, half h):
  - global attention: queries = its half (SH rows), keys/values = full seq
  - local windowed attention: fully contained in its half
  - output projections (g+l accumulated in PSUM) + residual + layernorm

All matmuls run fp8e4 with fp32 PSUM accumulation; contraction-128 matmuls
(projections, AV, output projection) use DoubleRow (2x effective throughput).
Scores (K=64 contraction) run plain fp8. Scaling ledger: weights host-scaled
x32 (fp8 subnormal avoidance), un-scaled in the PSUM->SBUF cast; attention
outputs carry x64 (ones-column = 1/64, so the rowsum reciprocal yields
64/sum); the final combine divides by 64*32 = 2048. Softmax skips the max
subtraction (scores are O(1)); rowsums ride the AV matmul via the ones
column (placed FIRST so the rowsum lands in psum partition 0 - the custom
DVE reciprocal ignores AP partition offsets); v dims sit at columns 64-127
(64-partition reads must start at partition 0 or 64).

Pipeline: the global-attention phase is scalar(exp)-bound, so all deferrable
tensor work - v projections, the six local projection sets, local attention
heads, and out-proj operand DMA loads - is emitted as fill units between
score/exp pairs of the global heads (emission order matters: engine queues
are in-order FIFOs, so every dependency of an instruction must be emitted
before it).
"""

import numpy as np
import ml_dtypes
from collections import deque
from contextlib import ExitStack

F8 = ml_dtypes.float8_e4m3

FULL_CFG = dict(S=2048, D=1024, H=16, K=64, NW=8)
N_CORES = 8
LN_EPS = 1e-3
WSCALE = 32.0    # host weight pre-scale
OSCALE = 64.0    # attention-output scale (ones column = 1/OSCALE)


def _chunks(total, size):
    return [(o, min(size, total - o)) for o in range(0, total, size)]


def flags_for(inputs):
    def nz(*keys):
        return any(np.any(np.asarray(inputs[k])) for k in keys)
    return dict(
        bqk_g=nz("gbq", "gbk"), bqk_l=nz("lbq", "lbk"),
        bv_g=nz("gbv"), bv_l=nz("lbv"),
        ln=(not np.all(np.asarray(inputs["gamma"]) == 1.0)
            or np.any(np.asarray(inputs["beta"]))),
    )


DEFAULT_FLAGS = dict(bqk_g=False, bqk_l=False, bv_g=False, bv_l=False,
                     ln=False)


def build_nc(cfg=None, flags=None):
    """Build + compile the per-core Bass program (SPMD, same on all cores)."""
    import concourse.bass as bass
    import concourse.tile as tile
    import concourse.mybir as mybir
    from concourse import bacc

    cfg = dict(cfg or FULL_CFG)
    flags = dict(flags or DEFAULT_FLAGS)
    S, D, H, K, NW = cfg["S"], cfg["D"], cfg["H"], cfg["K"], cfg["NW"]
    HK = H * K
    SH = S // 2          # per-core query rows (half the sequence)
    WIN = S // NW        # local attention window
    NWH = SH // WIN      # windows in this core's half
    assert K == 64 and D % 256 == 0 and HK % 128 == 0
    assert SH % 128 == 0 and WIN % 128 == 0 and NWH * WIN == SH

    ND2 = D // 256       # d-pair tiles (DoubleRow contracts 256 at a time)
    NHK = HK // 128      # head-pair tiles (2 heads each)
    NST = S // 128       # s-tiles (full seq)
    NSP = NST // 2       # s-tile pairs (global)
    NQT = SH // 128      # q-tiles (half seq)
    NLP = SH // 256      # local s-tile pairs = windows in half
    NOT = NHK // 2       # hk-pair tiles for output proj

    f32 = mybir.dt.float32
    fp8 = mybir.dt.float8e4
    DR = mybir.MatmulPerfMode.DoubleRow
    Exp = mybir.ActivationFunctionType.Exp
    Square = mybir.ActivationFunctionType.Square
    Sqrt = mybir.ActivationFunctionType.Sqrt
    add_op = mybir.AluOpType.add
    mult_op = mybir.AluOpType.mult
    sub_op = mybir.AluOpType.subtract

    nc = bacc.Bacc("TRN2", target_bir_lowering=False, debug=False,
                   num_devices=N_CORES)

    # ---- DRAM parameters -------------------------------------------------
    xT2_d = nc.dram_tensor("xT2", [ND2, 128, 2, S], fp8, kind="ExternalInput")
    xq_d = nc.dram_tensor("xq", [SH, D], f32, kind="ExternalInput")
    w_d = {}
    for nm in ("wq_g", "wk_g", "wv_g", "wq_l", "wk_l", "wv_l"):
        w_d[nm] = nc.dram_tensor(nm, [ND2, 128, 2, HK], fp8,
                                 kind="ExternalInput")
    wo_d = {st: nc.dram_tensor(f"wo_{st}", [NOT, 128, 2, D], fp8,
                               kind="ExternalInput") for st in ("g", "l")}
    bcol_d = {}
    for st in ("g", "l"):
        if flags[f"bqk_{st}"]:
            for qk in ("q", "k"):
                bcol_d[f"b{qk}_{st}"] = nc.dram_tensor(
                    f"b{qk}_{st}", [NHK, 128], f32, kind="ExternalInput")
        if flags[f"bv_{st}"]:
            bcol_d[f"bv_{st}"] = nc.dram_tensor(
                f"bv_{st}", [1, HK], fp8, kind="ExternalInput")
    if flags["ln"]:
        gamma_d = nc.dram_tensor("gamma", [1, D], f32, kind="ExternalInput")
        beta_d = nc.dram_tensor("beta", [1, D], f32, kind="ExternalInput")
    out_d = nc.dram_tensor("out", [SH, D], f32, kind="ExternalOutput")

    # DRAM scratch for attention outputs, laid out for the out-proj DoubleRow
    # load: [t, p, j, q] with hk = 256 t + 128 j + p
    oscr = {st: nc.dram_tensor(f"oscr_{st}", [NOT, 128, 2, SH], fp8)
            for st in ("g", "l")}

    PS = bass.MemorySpace.PSUM

    with tile.TileContext(nc) as tc, ExitStack() as ctx:
        # ---- whole-life pools -------------------------------------------
        cpool = ctx.enter_context(tc.tile_pool(name="consts", bufs=1))
        wopool = ctx.enter_context(tc.tile_pool(name="wo", bufs=1))
        odpool = ctx.enter_context(tc.tile_pool(name="od", bufs=1))

        ones8 = cpool.tile([1, 128], fp8, tag="ones", name="ones")
        nc.vector.memset(ones8[:], 1.0)
        eps_col = cpool.tile([128, 1], f32, tag="eps", name="eps")
        nc.vector.memset(eps_col[:], float(LN_EPS))
        bcol_sb = {}
        for nm, d in bcol_d.items():
            if nm.startswith("bv"):
                t = cpool.tile([1, HK], fp8, tag=nm, name=nm)
                nc.sync.dma_start(t[:], d[:])
                bcol_sb[nm] = t
            else:
                cols = []
                for j in range(NHK):
                    t = cpool.tile([128, 1], f32, tag=f"{nm}{j}",
                                   name=f"{nm}{j}")
                    nc.sync.dma_start(t[:], d[j, :].rearrange("(a b) -> a b",
                                                              b=1))
                    cols.append(t)
                bcol_sb[nm] = cols

        wo_sb = {}
        for st in ("g", "l"):
            wo_sb[st] = [wopool.tile([128, 2, D], fp8, tag=f"wo{st}{t}",
                                     name=f"wo{st}{t}") for t in range(NOT)]
        o_sb = {st: [odpool.tile([128, 2, SH], fp8, tag=f"ob{st}{t}",
                                 name=f"ob{st}{t}") for t in range(NOT)]
                for st in ("g", "l")}

        # ---- mid-life pools (released before phase D) --------------------
        mid_ctx = ctx.enter_context(ExitStack())
        kqv = mid_ctx.enter_context(tc.tile_pool(name="kqv", bufs=1))
        xin = mid_ctx.enter_context(tc.tile_pool(name="xin", bufs=1))
        wpool = mid_ctx.enter_context(tc.tile_pool(name="wt", bufs=1))

        kT_g = [kqv.tile([128, S], fp8, tag=f"ktg{j}", name=f"ktg{j}")
                for j in range(NHK)]
        qT_g = [kqv.tile([128, SH], fp8, tag=f"qtg{j}", name=f"qtg{j}")
                for j in range(NHK)]
        vx_g = [kqv.tile([128, 2, H, 128], fp8, tag=f"vxg{u}", name=f"vxg{u}")
                for u in range(NSP)]
        kT_l = [kqv.tile([128, SH], fp8, tag=f"ktl{j}", name=f"ktl{j}")
                for j in range(NHK)]
        qT_l = [kqv.tile([128, SH], fp8, tag=f"qtl{j}", name=f"qtl{j}")
                for j in range(NHK)]
        vx_l = [kqv.tile([128, 2, H, 128], fp8, tag=f"vxl{u}", name=f"vxl{u}")
                for u in range(NLP)]
        for u in range(NSP):
            nc.vector.memset(vx_g[u][:, :, :, 0:1], 1.0 / OSCALE)
        for u in range(NLP):
            nc.vector.memset(vx_l[u][:, :, :, 0:1], 1.0 / OSCALE)

        x2t = [xin.tile([128, 2, S], fp8, tag=f"xt{t}", name=f"xt{t}")
               for t in range(ND2)]
        w_sb = {}
        for nm in ("wk_g", "wq_g", "wv_g", "wk_l", "wq_l", "wv_l"):
            w_sb[nm] = [wpool.tile([128, 2, HK], fp8, tag=f"{nm}{t}",
                                   name=f"{nm}{t}") for t in range(ND2)]
        # DMA order: interleave x tiles with the first weight set so the
        # first projection chain starts as early as possible
        for t in range(ND2):
            nc.sync.dma_start(x2t[t][:], xT2_d[t])
            nc.sync.dma_start(w_sb["wk_g"][t][:], w_d["wk_g"][t])
        for nm in ("wq_g", "wv_g", "wk_l", "wq_l", "wv_l"):
            for t in range(ND2):
                nc.sync.dma_start(w_sb[nm][t][:], w_d[nm][t])
        for st in ("g", "l"):
            for t in range(NOT):
                nc.sync.dma_start(wo_sb[st][t][:], wo_d[st][t])

        # ================= Phase A: global K/Q projections ================
        with tc.tile_pool(name="ppA", bufs=2, space=PS) as ppA:
            def proj_kq_g(nm, s_len, out_tiles, bias):
                for j in range(NHK):
                    pt = ppA.tile([128, 2048], f32, tag="pp", name=f"p{nm}{j}")
                    for t in range(ND2):
                        for so, sl in _chunks(s_len, 512):
                            nc.tensor.matmul(
                                pt[:, so:so + sl],
                                w_sb[nm][t][:, :, j * 128:(j + 1) * 128],
                                x2t[t][:, :, so:so + sl],
                                start=(t == 0), stop=(t == ND2 - 1),
                                perf_mode=DR)
                    if bias is not None:
                        nc.vector.tensor_scalar(
                            out_tiles[j][:], pt[:, 0:s_len], 1.0 / WSCALE,
                            bias[j], mult_op, add_op)
                    else:
                        nc.vector.tensor_scalar(
                            out_tiles[j][:], pt[:, 0:s_len], 1.0 / WSCALE,
                            None, mult_op)

            proj_kq_g("wk_g", S, kT_g, bcol_sb.get("bk_g"))
            proj_kq_g("wq_g", SH, qT_g, bcol_sb.get("bq_g"))

        # ============== Phases B/C: attention + deferred fill work ========
        bc_ctx = ctx.enter_context(ExitStack())
        scp = bc_ctx.enter_context(tc.tile_pool(name="scp", bufs=2, space=PS))
        opp = bc_ctx.enter_context(tc.tile_pool(name="opp", bufs=2, space=PS))
        exp_p = bc_ctx.enter_context(tc.tile_pool(name="exp", bufs=3))
        nop = bc_ctx.enter_context(tc.tile_pool(name="nop", bufs=2))

        # --- fill units: projections into scp-pool psum tiles -------------
        def fill_kq(nm, out_tiles, bias, j):
            pt = scp.tile([128, SH], f32, tag="sc", name=f"f{nm}{j}")
            for t in range(ND2):
                for so, sl in _chunks(SH, 512):
                    nc.tensor.matmul(
                        pt[:, so:so + sl],
                        w_sb[nm][t][:, :, j * 128:(j + 1) * 128],
                        x2t[t][:, :, so:so + sl],
                        start=(t == 0), stop=(t == ND2 - 1), perf_mode=DR)
            if bias is not None:
                nc.vector.tensor_scalar(out_tiles[j][:], pt[:], 1.0 / WSCALE,
                                        bias[j], mult_op, add_op)
            else:
                nc.vector.tensor_scalar(out_tiles[j][:], pt[:], 1.0 / WSCALE,
                                        None, mult_op)

        def fill_v(nm, vx_tiles, bias_row, ts_):
            u, jj = divmod(ts_, 2)
            pt = scp.tile([128, SH], f32, tag="sc", name=f"f{nm}{ts_}")
            for t in range(ND2):
                for ho, hl in _chunks(HK, 512):
                    st_ = t == 0
                    sp_ = t == ND2 - 1 and bias_row is None
                    nc.tensor.matmul(
                        pt[:, ho:ho + hl],
                        x2t[t][:, :, ts_ * 128:(ts_ + 1) * 128],
                        w_sb[nm][t][:, :, ho:ho + hl],
                        start=st_, stop=sp_, perf_mode=DR)
            if bias_row is not None:
                for ho, hl in _chunks(HK, 512):
                    nc.tensor.matmul(pt[:, ho:ho + hl], ones8[0:1, 0:128],
                                     bias_row[0:1, ho:ho + hl],
                                     start=False, stop=True)
            nc.vector.tensor_scalar(
                vx_tiles[u][:, jj, :, 64:128],
                pt[:].rearrange("p (h k) -> p h k", k=64),
                1.0 / WSCALE, None, mult_op)

        def normalize_store(h, o_ps, dst):
            # rinv = OSCALE / rowsum ; o8 = o * rinv  (x OSCALE into fp8)
            rinv = nop.tile([1, SH], f32, tag="ri", name=f"ri{h}", bufs=1)
            nc.vector.reciprocal_approx_fast(out=rinv[:], in_=o_ps[0:1, :])
            rb = nop.tile([64, SH], f32, tag="rb", name=f"rb{h}")
            nc.gpsimd.partition_broadcast(rb[:], rinv[0:1, :])
            o8 = nop.tile([64, SH], fp8, tag="o8", name=f"o8{h}")
            nc.vector.tensor_tensor(o8[:], o_ps[64:128, :], rb[:], mult_op)
            t, j, pr = h // 4, (h % 4) // 2, 64 * (h % 2)
            nc.sync.dma_start(dst[t][pr:pr + 64, j, :], o8[:])

        def local_head(h):
            hp, po = h // 2, 64 * (h % 2)
            o_ps = opp.tile([128, SH], f32, tag="o", name=f"ol{h}")
            ex2 = exp_p.tile([128, 2, SH], fp8, tag="ex", name=f"exl{h}")
            for ss in range(2):
                sc = scp.tile([128, SH], f32, tag="sc", name=f"scl{h}{ss}")
                for w in range(NWH):
                    st = 2 * w + ss
                    nc.tensor.matmul(
                        sc[:, w * WIN:(w + 1) * WIN],
                        kT_l[hp][po:po + 64, st * 128:(st + 1) * 128],
                        qT_l[hp][po:po + 64, w * WIN:(w + 1) * WIN],
                        start=True, stop=True)
                nc.scalar.activation(ex2[:, ss, :], sc[:], Exp, scale=0.125)
            for w in range(NWH):
                nc.tensor.matmul(
                    o_ps[:, w * WIN:(w + 1) * WIN], vx_l[w][:, :, h, :],
                    ex2[:, :, w * WIN:(w + 1) * WIN],
                    start=(w % 2 == 0), stop=(w % 2 == 1), perf_mode=DR)
            normalize_store(h, o_ps, oscr["l"])

        def load_o2(st, t):
            nc.sync.dma_start(o_sb[st][t][:], oscr[st][t])

        # fill queue: (cost_estimate_us, closure); emitted between score/exp
        # pairs. Order respects the in-order engine FIFOs: everything an
        # emitted instruction depends on is emitted earlier. v_g fills are
        # kept separate: ensure_vg() guarantees the producer of vx_g[u] is
        # emitted before any AV matmul that reads it.
        vg_next = [0]

        def ensure_vg(up_to_ts):
            while vg_next[0] <= min(up_to_ts, NST - 1):
                fill_v("wv_g", vx_g, bcol_sb.get("bv_g"), vg_next[0])
                vg_next[0] += 1

        fills = deque()
        for j in range(NHK):
            fills.append((1.8, lambda j=j: fill_kq(
                "wk_l", kT_l, bcol_sb.get("bk_l"), j)))
        for j in range(NHK):
            fills.append((1.8, lambda j=j: fill_kq(
                "wq_l", qT_l, bcol_sb.get("bq_l"), j)))
        for ts_ in range(SH // 128):
            fills.append((1.8, lambda ts_=ts_: fill_v(
                "wv_l", vx_l, bcol_sb.get("bv_l"), ts_)))

        for h in range(H):
            fills.append((2.2, lambda h=h: local_head(h)))
            if h % 4 == 3:
                fills.append((0.0, lambda t=h // 4: load_o2("l", t)))

        def pop_fills(budget):
            while fills and budget > 0:
                cost, fn = fills.popleft()
                fn()
                budget -= max(cost, 0.1)

        # v_g pairs 0-1 must exist before head 0's first AV matmuls
        ensure_vg(3)

        # --- global attention, one head at a time -------------------------
        for h in range(H):
            hp, po = h // 2, 64 * (h % 2)
            o_ps = opp.tile([128, SH], f32, tag="o", name=f"og{h}")
            pend = deque()

            def do_av(item, o_ps=o_ps, h=h):
                ex2, u = item
                ensure_vg(2 * u + 1)
                for qo, ql in _chunks(SH, 512):
                    nc.tensor.matmul(
                        o_ps[:, qo:qo + ql], vx_g[u][:, :, h, :],
                        ex2[:, :, qo:qo + ql],
                        start=(u == 0), stop=(u == NSP - 1), perf_mode=DR)

            for u in range(NSP):
                ex2 = exp_p.tile([128, 2, SH], fp8, tag="ex",
                                 name=f"exg{h}{u}")
                for j in range(2):
                    st = 2 * u + j
                    sc = scp.tile([128, SH], f32, tag="sc", name=f"scg{h}{st}")
                    for qo, ql in _chunks(SH, 512):
                        nc.tensor.matmul(
                            sc[:, qo:qo + ql],
                            kT_g[hp][po:po + 64, st * 128:(st + 1) * 128],
                            qT_g[hp][po:po + 64, qo:qo + ql],
                            start=True, stop=True)
                    nc.scalar.activation(ex2[:, j, :], sc[:], Exp, scale=0.125)
                pend.append((ex2, u))
                if len(pend) > 1:
                    do_av(pend.popleft())
                pop_fills(1.0)
            while pend:
                do_av(pend.popleft())
            ensure_vg(NST - 1)
            normalize_store(h, o_ps, oscr["g"])
            if h % 4 == 3:
                load_o2("g", h // 4)

        while fills:
            cost, fn = fills.popleft()
            fn()


        # ========== Phase D: output projection + residual + layernorm ====
        bc_ctx.close()   # release attention PSUM banks for the ypp pool
        mid_ctx.close()  # release K/Q/V + x + projection-weight SBUF

        with tc.tile_pool(name="ypp", bufs=2, space=PS) as ypp, \
             tc.tile_pool(name="ln", bufs=2) as lnp:
            if flags["ln"]:
                gamma_bc = lnp.tile([128, D], f32, tag="gamma", name="gamma",
                                    bufs=1)
                nc.sync.dma_start(gamma_bc[:],
                                  gamma_d[:].partition_broadcast(128))
                beta_bc = lnp.tile([128, D], f32, tag="beta", name="beta",
                                   bufs=1)
                nc.sync.dma_start(beta_bc[:],
                                  beta_d[:].partition_broadcast(128))
            for qt in range(NQT):
                xq_t = lnp.tile([128, D], f32, tag="xq", name=f"xq{qt}")
                nc.sync.dma_start(xq_t[:], xq_d[qt * 128:(qt + 1) * 128, :])
                ps_y = ypp.tile([128, D], f32, tag="py", name=f"py{qt}")
                for do, dl in _chunks(D, 512):
                    first = True
                    for st in ("g", "l"):
                        for t in range(NOT):
                            nc.tensor.matmul(
                                ps_y[:, do:do + dl],
                                o_sb[st][t][:, :, qt * 128:(qt + 1) * 128],
                                wo_sb[st][t][:, :, do:do + dl],
                                start=first, stop=(st == "l" and t == NOT - 1),
                                perf_mode=DR)
                            first = False
                y = lnp.tile([128, D], f32, tag="y", name=f"y{qt}")
                ssum = lnp.tile([128, 1], f32, tag="ssum", name=f"ssum{qt}")
                nc.vector.scalar_tensor_tensor(
                    y[:], ps_y[:], 1.0 / (WSCALE * OSCALE), xq_t[:],
                    mult_op, add_op, accum_out=ssum[:])
                sqd = lnp.tile([128, D], f32, tag="sqd", name=f"sqd{qt}")
                ssq = lnp.tile([128, 1], f32, tag="ssq", name=f"ssq{qt}")
                nc.scalar.activation(sqd[:], y[:], Square, accum_out=ssq[:])
                mu = lnp.tile([128, 1], f32, tag="mu", name=f"mu{qt}")
                nc.vector.tensor_scalar_mul(mu[:], ssum[:], 1.0 / D)
                var = lnp.tile([128, 1], f32, tag="var", name=f"var{qt}")
                nc.vector.tensor_scalar_mul(var[:], ssq[:], 1.0 / D)
                mu2 = lnp.tile([128, 1], f32, tag="mu2", name=f"mu2{qt}")
                nc.vector.tensor_tensor(mu2[:], mu[:], mu[:], mult_op)
                nc.vector.tensor_tensor(var[:], var[:], mu2[:], sub_op)
                sd = lnp.tile([128, 1], f32, tag="sd", name=f"sd{qt}")
                nc.scalar.activation(sd[:], var[:], Sqrt, bias=eps_col[:])
                rstd = lnp.tile([128, 1], f32, tag="rstd", name=f"rstd{qt}")
                nc.vector.reciprocal(rstd[:], sd[:])
                bco = lnp.tile([128, 1], f32, tag="bco", name=f"bco{qt}")
                nc.vector.tensor_tensor(bco[:], mu[:], rstd[:], mult_op)
                nc.vector.tensor_scalar_mul(bco[:], bco[:], -1.0)
                ot = lnp.tile([128, D], f32, tag="ot", name=f"ot{qt}")
                nc.vector.tensor_scalar(ot[:], y[:], rstd[:], bco[:],
                                        mult_op, add_op)
                if flags["ln"]:
                    t2 = lnp.tile([128, D], f32, tag="t2", name=f"t2{qt}")
                    nc.vector.tensor_tensor(t2[:], ot[:], gamma_bc[:], mult_op)
                    nc.vector.tensor_tensor(ot[:], t2[:], beta_bc[:], add_op)
                nc.sync.dma_start(out_d[qt * 128:(qt + 1) * 128, :], ot[:])

    nc.compile()
    return nc


def make_in_maps(inputs, cfg=None, flags=None):
    """Build per-core input maps from the full (unsharded) problem inputs."""
    cfg = dict(cfg or FULL_CFG)
    flags = dict(flags or DEFAULT_FLAGS)
    S, D, H, K = cfg["S"], cfg["D"], cfg["H"], cfg["K"]
    HK = H * K
    SH = S // 2
    ND2 = D // 256
    NHK = HK // 128
    NOT = NHK // 2

    def np32(a):
        return np.asarray(a, dtype=np.float32)

    def dpair(w):  # [D, X] -> [ND2, 128, 2, X]
        return np.ascontiguousarray(
            w.reshape(ND2, 2, 128, -1).transpose(0, 2, 1, 3))

    shared = {}
    for nm, key in (("wq_g", "gWq"), ("wk_g", "gWk"), ("wv_g", "gWv"),
                    ("wq_l", "lWq"), ("wk_l", "lWk"), ("wv_l", "lWv")):
        shared[nm] = dpair(np32(inputs[key]).reshape(D, HK) * WSCALE).astype(F8)
    for st, key in (("g", "gWo"), ("l", "lWo")):
        w = np32(inputs[key]).reshape(HK, D) * WSCALE
        shared[f"wo_{st}"] = np.ascontiguousarray(
            w.reshape(NOT, 2, 128, D).transpose(0, 2, 1, 3)).astype(F8)
    for st, q, k in (("g", "gbq", "gbk"), ("l", "lbq", "lbk")):
        if flags[f"bqk_{st}"]:
            shared[f"bq_{st}"] = np.ascontiguousarray(
                np32(inputs[q]).reshape(NHK, 128))
            shared[f"bk_{st}"] = np.ascontiguousarray(
                np32(inputs[k]).reshape(NHK, 128))
    for st, key in (("g", "gbv"), ("l", "lbv")):
        if flags[f"bv_{st}"]:
            shared[f"bv_{st}"] = (np32(inputs[key]).reshape(1, HK)
                                  * WSCALE).astype(F8)
    if flags["ln"]:
        shared["gamma"] = np32(inputs["gamma"]).reshape(1, D)
        shared["beta"] = np32(inputs["beta"]).reshape(1, D)

    x = np32(inputs["x"])
    bo = np32(inputs["gbo"]) + np32(inputs["lbo"])
    in_maps = []
    for c in range(N_CORES):
        b, half = divmod(c, 2)
        xb = x[b]
        # own half first (queries/local), other half second; global attention
        # is invariant to key/value column order
        xperm = np.concatenate([xb[half * SH:(half + 1) * SH],
                                xb[(1 - half) * SH:(2 - half) * SH]], axis=0)
        m = dict(shared)
        m["xT2"] = dpair(np.ascontiguousarray(xperm.T)).astype(F8)
        m["xq"] = np.ascontiguousarray(xperm[0:SH]) + bo
        in_maps.append(m)
    return in_maps


def assemble_out(results, cfg=None):
    cfg = dict(cfg or FULL_CFG)
    S, D = cfg["S"], cfg["D"]
    SH = S // 2
    B = N_CORES // 2
    out = np.empty((B, S, D), np.float32)
    for c in range(N_CORES):
        b, half = divmod(c, 2)
        out[b, half * SH:(half + 1) * SH] = results[c]["out"]
    return out


_NC_CACHE = {}


def kernel(**inputs):
    from concourse.bass_utils import run_bass_kernel_spmd
    flags = flags_for(inputs)
    key = tuple(sorted(flags.items()))
    if key not in _NC_CACHE:
        _NC_CACHE[key] = build_nc(flags=flags)
    nc = _NC_CACHE[key]
    in_maps = make_in_maps(inputs, flags=flags)
    res = run_bass_kernel_spmd(nc, in_maps, list(range(N_CORES)))
    return assemble_out(res.results)
